# revision 1
# baseline (speedup 1.0000x reference)
"""Trainium2 Bass kernel for nn_MeshGenLoss (Chamfer + KL + density + sizing + material).

Strategy (8 NeuronCores, SPMD one NEFF; ~350 us HW exec, rel err ~4e-7):
  - Shard: core c = 2*b + h handles batch b, pred rows [h*4096, (h+1)*4096), all 8192 targets.
  - Chamfer pairwise distances via an augmented matmul on TensorE:
        d(n,m) = |a_n|^2 + |b_m|^2 - 2 a.b  =  [x,y,z,|a|^2,1]_n . [-2x,-2y,-2z,1,|b|^2]_m
    run in float32r (1 cycle/row vs 4 for fp32). f32r is only exact for
    fp16-representable operands, so every augmented row is pre-split on the host
    into an fp16-exact (hi, lo) pair and the contraction expanded to K=13 rows
    (hi*hi + hi*lo + lo*hi); K is free on the 128-deep systolic array, giving
    fp32-grade accuracy at full PE speed.
    ScalarE evacuates PSUM -> SBUF fp16; VectorE does 2x-mode fp16 min work:
      * row direction (min over m): pairwise-min fold chain + final short reduce
      * col direction (min over n): running elementwise-min accumulator, finished by
        PE transposes + free-axis reduces.
    Engine busy (cost model): DVE 310 us (bottleneck), ACT 248 us, PE 118 us.
  - Density (1024-pt subsample) via the same machinery; self-distance masked by adding
    a 30000-diag eye to the rolled-diagonal block (host rolls the subsample per core so
    each core's diagonal lands at compile-time-constant columns).
  - KL / sizing / material: tiny elementwise + reduce work, partials per core.
  - Host: trivial gather/combine of per-core partial vectors into the scalar loss.
"""

import numpy as np

import concourse.bacc as bacc
import concourse.mybir as mybir
import concourse.tile as tile
from concourse.bass_utils import run_bass_kernel_spmd

P = 128          # partitions
NSH = 4096       # pred rows per core
M = 8192         # target points
NT = NSH // P    # 32 row tiles
PSC = 2048       # psum chunk (4 banks of fp32)
NCH = M // PSC   # 4 chunks per row tile
MMF = 512        # matmul free dim (1 psum bank)
SUB = 1024       # density subsample size
DROWS = 512      # density rows per core
DT = DROWS // P  # 4 density row tiles
BIG = 30000.0    # > any real squared distance; fp16-safe
KA = 13          # split-row contraction: 3x(xyz) + 2x(|a|^2) + 2x(1)

F32 = mybir.dt.float32
F32R = mybir.dt.float32r
F16 = mybir.dt.float16
MIN = mybir.AluOpType.min

_CACHE = {}
TRACE = False


def _foldmin_to_rowcol(nc, pool, src, width, out_col, stop=128):
    """Reduce src[:, :width] (fp16) along free axis into out_col [P,1] via
    pairwise-min folds (2x mode) down to `stop`, then one 1x reduce."""
    f = pool.tile([P, width // 2], F16, tag="fold")
    nc.vector.tensor_tensor(
        out=f[:, :width // 2], in0=src[:, :width // 2], in1=src[:, width // 2:width],
        op=MIN,
    )
    w = width // 2
    while w > stop:
        nc.vector.tensor_tensor(
            out=f[:, :w // 2], in0=f[:, :w // 2], in1=f[:, w // 2:w], op=MIN,
        )
        w //= 2
    nc.vector.tensor_reduce(
        out=out_col, in_=f[:, :w], op=MIN, axis=mybir.AxisListType.X,
    )


def build_nc(reps: int = 1):
    nc = bacc.Bacc("TRN2", target_bir_lowering=False, debug=False, num_devices=8)

    paug = nc.dram_tensor("paug", [KA, NSH], F32R, kind="ExternalInput")
    taug = nc.dram_tensor("taug", [KA, M], F32R, kind="ExternalInput")
    dlaug = nc.dram_tensor("dlaug", [KA, DROWS], F32R, kind="ExternalInput")
    draug = nc.dram_tensor("draug", [KA, SUB], F32R, kind="ExternalInput")
    ident = nc.dram_tensor("ident", [P, P], F16, kind="ExternalInput")
    eyeb = nc.dram_tensor("eyeb", [P, P], F16, kind="ExternalInput")
    muh = nc.dram_tensor("muh", [P, 2], F32, kind="ExternalInput")
    lvh = nc.dram_tensor("lvh", [P, 2], F32, kind="ExternalInput")
    sizp = nc.dram_tensor("sizp", [P, 32], F32, kind="ExternalInput")
    sizt = nc.dram_tensor("sizt", [P, 32], F32, kind="ExternalInput")
    matp = nc.dram_tensor("matp", [P, 128], F32, kind="ExternalInput")
    matt = nc.dram_tensor("matt", [P, 128], F32, kind="ExternalInput")

    rowmins = nc.dram_tensor("rowmins", [P, NT], F32, kind="ExternalOutput")
    colmins = nc.dram_tensor("colmins", [P, M // P], F32, kind="ExternalOutput")
    dnn = nc.dram_tensor("dnn", [P, DT], F32, kind="ExternalOutput")
    klv = nc.dram_tensor("klv", [P, 2], F32, kind="ExternalOutput")
    ssum = nc.dram_tensor("ssum", [P, 1], F32, kind="ExternalOutput")
    msum = nc.dram_tensor("msum", [P, 1], F32, kind="ExternalOutput")

    with tile.TileContext(nc) as tc:
        with (
            tc.tile_pool(name="const", bufs=1) as cpool,
            tc.tile_pool(name="psum", bufs=2, space="PSUM") as pspool,
            tc.tile_pool(name="stage", bufs=3) as stpool,
            tc.tile_pool(name="acc", bufs=1) as accpool,
            tc.tile_pool(name="fold", bufs=2) as fpool,
            tc.tile_pool(name="outs", bufs=1) as opool,
        ):
            ident_sb = cpool.tile([P, P], F16)
            nc.sync.dma_start(out=ident_sb, in_=ident[:, :])
            eyeb_sb = cpool.tile([P, P], F16)
            nc.sync.dma_start(out=eyeb_sb, in_=eyeb[:, :])
            muh_sb = cpool.tile([P, 2], F32)
            nc.sync.dma_start(out=muh_sb, in_=muh[:, :])
            lvh_sb = cpool.tile([P, 2], F32)
            nc.sync.dma_start(out=lvh_sb, in_=lvh[:, :])
            sizp_sb = cpool.tile([P, 32], F32)
            nc.sync.dma_start(out=sizp_sb, in_=sizp[:, :])
            sizt_sb = cpool.tile([P, 32], F32)
            nc.sync.dma_start(out=sizt_sb, in_=sizt[:, :])
            matp_sb = cpool.tile([P, 128], F32)
            nc.sync.dma_start(out=matp_sb, in_=matp[:, :])
            matt_sb = cpool.tile([P, 128], F32)
            nc.sync.dma_start(out=matt_sb, in_=matt[:, :])
            dlaug_sb = cpool.tile([KA, DROWS], F32R)
            nc.sync.dma_start(out=dlaug_sb, in_=dlaug[:, :])
            draug_sb = cpool.tile([KA, SUB], F32R)
            nc.sync.dma_start(out=draug_sb, in_=draug[:, :])
            paug_sb = cpool.tile([KA, NSH], F32R)
            nc.sync.dma_start(out=paug_sb, in_=paug[:, :])
            taug_sb = cpool.tile([KA, M], F32R)
            nc.sync.dma_start(out=taug_sb, in_=taug[:, :])

            rowm_sb = opool.tile([P, NT], F32)
            colm_sb = opool.tile([P, M // P], F32)
            dnn_sb = opool.tile([P, DT], F32)

            # --- small terms first (fills ACT table-load / DMA-wait window) ---
            kl_e = opool.tile([P, 2], F32)
            nc.scalar.activation(
                out=kl_e, in_=lvh_sb, func=mybir.ActivationFunctionType.Exp
            )
            kl_t = opool.tile([P, 2], F32)
            nc.vector.tensor_mul(out=kl_t, in0=muh_sb, in1=muh_sb)
            nc.vector.tensor_sub(out=kl_t, in0=lvh_sb, in1=kl_t)
            nc.vector.tensor_sub(out=kl_t, in0=kl_t, in1=kl_e)
            nc.sync.dma_start(out=klv[:, :], in_=kl_t)

            sd = opool.tile([P, 32], F32)
            nc.vector.tensor_sub(out=sd, in0=sizp_sb, in1=sizt_sb)
            nc.vector.tensor_mul(out=sd, in0=sd, in1=sd)
            ssum_sb = opool.tile([P, 1], F32)
            nc.vector.reduce_sum(out=ssum_sb, in_=sd, axis=mybir.AxisListType.X)
            nc.sync.dma_start(out=ssum[:, :], in_=ssum_sb)

            md = opool.tile([P, 128], F32)
            nc.vector.tensor_sub(out=md, in0=matp_sb, in1=matt_sb)
            nc.vector.tensor_mul(out=md, in0=md, in1=md)
            msum_sb = opool.tile([P, 1], F32)
            nc.vector.reduce_sum(out=msum_sb, in_=md, axis=mybir.AxisListType.X)
            nc.sync.dma_start(out=msum[:, :], in_=msum_sb)

            # --- density: 4 row tiles x 1024 cols ---
            for t in range(DT):
                dps = pspool.tile([P, SUB], F32, tag="ps")
                for s in range(SUB // MMF):
                    nc.tensor.matmul(
                        dps[:, s * MMF:(s + 1) * MMF],
                        dlaug_sb[:, t * P:(t + 1) * P],
                        draug_sb[:, s * MMF:(s + 1) * MMF],
                        start=True, stop=True,
                    )
                dstage = fpool.tile([P, SUB], F16, tag="dstage")
                nc.scalar.copy(out=dstage, in_=dps)
                nc.vector.tensor_add(
                    out=dstage[:, t * P:(t + 1) * P],
                    in0=dstage[:, t * P:(t + 1) * P],
                    in1=eyeb_sb,
                )
                _foldmin_to_rowcol(nc, fpool, dstage, SUB, dnn_sb[:, t:t + 1])
            nc.sync.dma_start(out=dnn[:, :], in_=dnn_sb)

            # --- chamfer main loop ---
            colacc = accpool.tile([P, M], F16)

            for _rep in range(reps):
              for i in range(NT):
                  stage = stpool.tile([P, M], F16, tag="stage")
                  for c in range(NCH):
                      ps = pspool.tile([P, PSC], F32, tag="ps")
                      for s in range(PSC // MMF):
                          j0 = c * PSC + s * MMF
                          nc.tensor.matmul(
                              ps[:, s * MMF:(s + 1) * MMF],
                              paug_sb[:, i * P:(i + 1) * P],
                              taug_sb[:, j0:j0 + MMF],
                              start=True, stop=True,
                          )
                      nc.scalar.copy(out=stage[:, c * PSC:(c + 1) * PSC], in_=ps)
                      if i == 0:
                          nc.vector.tensor_copy(
                              colacc[:, c * PSC:(c + 1) * PSC],
                              stage[:, c * PSC:(c + 1) * PSC],
                          )
                  if i > 0:
                      nc.vector.tensor_tensor(out=colacc, in0=colacc, in1=stage, op=MIN)
                  _foldmin_to_rowcol(nc, fpool, stage, M, rowm_sb[:, i:i + 1])
            nc.sync.dma_start(out=rowmins[:, :], in_=rowm_sb)

            # --- colmin epilogue: cross-partition min via PE transpose ---
            for k in range(M // P):
                tp = pspool.tile([P, P], F16, tag="ps")
                nc.tensor.transpose(tp, colacc[:, k * P:(k + 1) * P], ident_sb)
                nc.vector.tensor_reduce(
                    out=colm_sb[:, k:k + 1], in_=tp, op=MIN,
                    axis=mybir.AxisListType.X,
                )
            nc.sync.dma_start(out=colmins[:, :], in_=colm_sb)

    nc.finalize()
    return nc


def _hl(v):
    # split fp32 -> (hi, lo), both exactly fp16-representable; v ~= hi + lo
    hi = v.astype(np.float16).astype(np.float32)
    lo = (v - hi).astype(np.float16).astype(np.float32)
    return hi, lo


def _aug_lhs(pts):
    # [n, 3] -> [KA, n] split-row layout paired with _aug_rhs:
    # per coord c: [c_h, c_h, c_l]; then [|p|^2_h, |p|^2_l]; then [1, 1]
    n = pts.shape[0]
    out = np.empty((KA, n), np.float32)
    for c in range(3):
        h, l = _hl(pts[:, c])
        out[3 * c] = h
        out[3 * c + 1] = h
        out[3 * c + 2] = l
    h, l = _hl((pts.astype(np.float64) ** 2).sum(axis=1).astype(np.float32))
    out[9] = h
    out[10] = l
    out[11] = 1.0
    out[12] = 1.0
    return out


def _aug_rhs(pts):
    # [m, 3] -> [KA, m]: per coord c of u=-2c: [u_h, u_l, u_h]; then [1, 1];
    # then [|p|^2_h, |p|^2_l]
    m = pts.shape[0]
    out = np.empty((KA, m), np.float32)
    for c in range(3):
        h, l = _hl(-2.0 * pts[:, c])
        out[3 * c] = h
        out[3 * c + 1] = l
        out[3 * c + 2] = h
    out[9] = 1.0
    out[10] = 1.0
    h, l = _hl((pts.astype(np.float64) ** 2).sum(axis=1).astype(np.float32))
    out[11] = h
    out[12] = l
    return out


def _make_in_maps(pred_pos, pred_sizing, pred_material, target_pos,
                  target_sizing, target_material, mu, logvar, sub_idx):
    B, N, _ = pred_pos.shape
    L = mu.shape[1]
    identity = np.eye(P, dtype=np.float16)
    eyebig = (BIG * np.eye(P)).astype(np.float16)

    in_maps = []
    for c in range(8):
        b, h = divmod(c, 2)
        psh = pred_pos[b, h * NSH:(h + 1) * NSH]
        pts = pred_pos[b][sub_idx]                       # [1024, 3]
        rolled = np.roll(pts, -h * DROWS, axis=0)
        in_maps.append({
            "paug": np.ascontiguousarray(_aug_lhs(psh)),
            "taug": np.ascontiguousarray(_aug_rhs(target_pos[b])),
            "dlaug": np.ascontiguousarray(_aug_lhs(rolled[:DROWS])),
            "draug": np.ascontiguousarray(_aug_rhs(rolled)),
            "ident": identity,
            "eyeb": eyebig,
            "muh": np.ascontiguousarray(mu[b, h * L // 2:(h + 1) * L // 2].reshape(P, 2)),
            "lvh": np.ascontiguousarray(logvar[b, h * L // 2:(h + 1) * L // 2].reshape(P, 2)),
            "sizp": np.ascontiguousarray(pred_sizing[b, h * NSH:(h + 1) * NSH, 0].reshape(P, 32)),
            "sizt": np.ascontiguousarray(target_sizing[b, h * NSH:(h + 1) * NSH, 0].reshape(P, 32)),
            "matp": np.ascontiguousarray(pred_material[b, h * NSH:(h + 1) * NSH].reshape(P, 128)),
            "matt": np.ascontiguousarray(target_material[b, h * NSH:(h + 1) * NSH].reshape(P, 128)),
        })
    return in_maps


def kernel(pred_pos, pred_sizing, pred_material, target_pos,
           target_sizing, target_material, mu, logvar, sub_idx):
    pred_pos = np.asarray(pred_pos, np.float32)
    pred_sizing = np.asarray(pred_sizing, np.float32)
    pred_material = np.asarray(pred_material, np.float32)
    target_pos = np.asarray(target_pos, np.float32)
    target_sizing = np.asarray(target_sizing, np.float32)
    target_material = np.asarray(target_material, np.float32)
    mu = np.asarray(mu, np.float32)
    logvar = np.asarray(logvar, np.float32)
    sub_idx = np.asarray(sub_idx)

    B, N, _ = pred_pos.shape
    L = mu.shape[1]

    reps = int(_CACHE.get("reps", 1))
    key = f"nc{reps}"
    if key not in _CACHE:
        _CACHE[key] = build_nc(reps)
    nc = _CACHE[key]

    in_maps = _make_in_maps(pred_pos, pred_sizing, pred_material, target_pos,
                            target_sizing, target_material, mu, logvar, sub_idx)

    trace_kw = {}
    if TRACE:
        trace_kw = dict(trace=True, trace_cores=list(range(8)), stitch_traces=True)
    res = run_bass_kernel_spmd(nc, in_maps, core_ids=list(range(8)), **trace_kw)
    outs = res.results
    _CACHE["last_res"] = res

    # --- host combine (small vectors only) ---
    cd = 0.0
    density = 0.0
    for b in range(B):
        o0, o1 = outs[2 * b], outs[2 * b + 1]
        rm = np.concatenate([
            o0["rowmins"].T.reshape(-1), o1["rowmins"].T.reshape(-1)
        ]).astype(np.float64)
        cm = np.minimum(o0["colmins"], o1["colmins"]).T.reshape(-1).astype(np.float64)
        cd += rm.mean() + cm.mean()
        nn_d = np.concatenate([
            o0["dnn"].T.reshape(-1), o1["dnn"].T.reshape(-1)
        ]).astype(np.float64)
        density += nn_d.std(ddof=1)
    cd /= B
    density /= B

    klt = sum(float(o["klv"].astype(np.float64).sum()) for o in outs)
    kl = -0.5 * (1.0 + klt / (B * L))

    s_sq = sum(float(o["ssum"].astype(np.float64).sum()) for o in outs)
    sizing = s_sq / (B * N * 1)
    m_sq = sum(float(o["msum"].astype(np.float64).sum()) for o in outs)
    material = m_sq / (B * N * 4)

    total = cd + 0.001 * kl + 0.1 * density + 0.05 * sizing + 0.1 * material
    return np.float32(total)

